# revision 2
# baseline (speedup 1.0000x reference)
"""Trainium2 Bass kernel for nn_KalmanFilter (B=2048, T=200, NS=16, MS=8).

Key structure: the Kalman covariance recursion (P_t, S_t, K_t, Sig_t) is
data-independent and the initial covariance is shared by every series, so the
whole covariance path is identical across the batch.  The host computes that
tiny 199-step recursion once (a few hundred KB of derived parameters); the
device then does the actual data-dependent work:

  * the batched mean recursion  m_t = A_t m_{t-1} + B_t y_{t-1}  (with
    mu_t = H m_t stacked into the same matmul pair) on the PE array, and
  * writing the full ~563MB of outputs, which is what makes this problem
    memory-bound: the covariance outputs are broadcast-writes of the shared
    200-step path, done as large (0.8-3.3MB) contiguous DMAs from replicated
    SBUF tiles.

Sharding: data-parallel over batch, 256 series per core on 8 cores.
"""

import os

import numpy as np

NS, MS = 16, 8
B, T = 2048, 200
N_CORES = 8
BC = B // N_CORES          # 256 series per core
TS = T - 1                 # 199 recursion steps
OUTW = NS + MS             # 24 stacked output rows (m_t ; mu_t)

# Covariance broadcast-write geometry: one DMA writes `GROUP_ROWS` batch rows
# of the (T,NS,NS) / (T,MS,MS) path from a [128, *] SBUF tile.
N_GROUPS = 16
GROUP_ROWS = BC // N_GROUPS            # 16 batch rows per DMA
CS_ROW = T * NS * NS                   # 51200 floats per batch row
CM_ROW = T * MS * MS                   # 12800 floats per batch row
CS_COL = GROUP_ROWS * CS_ROW // 128    # 6400 floats per partition
CM_COL = GROUP_ROWS * CM_ROW // 128    # 1600 floats per partition
REP = 128 * CS_COL // CS_ROW           # 16 partition-replicas of the 8-row base

CHUNK_T = 16               # mean-output staging chunk (steps per DMA)
YCHUNK = 8                 # observation steps per input DMA

_CACHE = {}


def _host_precompute(init_state_mean, init_log_diag, init_off_diag, F, H, Q, R):
    """Mirror the reference's float32 covariance recursion; derive the
    stacked affine-step matrices for the device mean recursion."""
    f32 = np.float32
    F = np.asarray(F, f32)
    H = np.asarray(H, f32)
    Q = np.asarray(Q, f32)
    R = np.asarray(R, f32)
    m0 = np.asarray(init_state_mean, f32)
    log_diag = np.asarray(init_log_diag, f32)
    off_diag = np.asarray(init_off_diag, f32)

    L0 = np.zeros((NS, NS), f32)
    L0[np.tril_indices(NS, -1)] = off_diag
    L0 += np.diag(np.exp(log_diag))
    P0 = (L0 @ L0.T).astype(f32)
    I = np.eye(NS, dtype=f32)

    P_seq = np.empty((T, NS, NS), f32)
    Sig_seq = np.empty((T, MS, MS), f32)
    LA = np.empty((TS, NS, OUTW), f32)   # lhsT for [A_t; H A_t]
    LB = np.empty((TS, MS, OUTW), f32)   # lhsT for [B_t; H B_t]

    P = P0
    P_seq[0] = P0
    Sig_seq[0] = H @ P0 @ H.T + R
    for s in range(1, T):
        HP = H @ P                                   # (MS, NS)
        S = HP @ H.T + R
        K = np.linalg.solve(S, HP).T.astype(f32)     # (NS, MS) = P H^T S^-1
        ImKH = (I - K @ H).astype(f32)
        A = (F @ ImKH).astype(f32)
        Bm = (F @ K).astype(f32)
        P = (F @ (ImKH @ P) @ F.T + Q).astype(f32)
        P_seq[s] = P
        Sig_seq[s] = H @ P @ H.T + R
        W = np.concatenate([A, H @ A], axis=0)       # (OUTW, NS)
        V = np.concatenate([Bm, H @ Bm], axis=0)     # (OUTW, MS)
        LA[s - 1] = W.T
        LB[s - 1] = V.T

    mu0 = (m0 @ H.T).astype(f32)
    return {
        "LA": np.ascontiguousarray(LA.transpose(1, 0, 2).reshape(NS, TS * OUTW)),
        "LB": np.ascontiguousarray(LB.transpose(1, 0, 2).reshape(MS, TS * OUTW)),
        "PREP": P_seq.reshape(8, CS_COL).copy(),
        "SREP": Sig_seq.reshape(8, CM_COL).copy(),
        "M0": np.ascontiguousarray(np.broadcast_to(m0[:, None], (NS, BC))),
        "m0": m0,
        "mu0": mu0,
    }


def _build_nc():
    import concourse.bacc as bacc
    import concourse.tile as tile
    from concourse import mybir

    nc = bacc.Bacc("TRN2", debug=True)
    f32 = mybir.dt.float32

    LA = nc.dram_tensor("LA", (NS, TS * OUTW), f32, kind="ExternalInput")
    LB = nc.dram_tensor("LB", (MS, TS * OUTW), f32, kind="ExternalInput")
    PREP = nc.dram_tensor("PREP", (8, CS_COL), f32, kind="ExternalInput")
    SREP = nc.dram_tensor("SREP", (8, CM_COL), f32, kind="ExternalInput")
    M0 = nc.dram_tensor("M0", (NS, BC), f32, kind="ExternalInput")
    Y = nc.dram_tensor("Y", (MS, TS * BC), f32, kind="ExternalInput")

    CS = nc.dram_tensor("CS", (N_GROUPS, 128, CS_COL), f32, kind="ExternalOutput")
    CM = nc.dram_tensor("CM", (N_GROUPS, 128, CM_COL), f32, kind="ExternalOutput")
    MR = nc.dram_tensor("MR", (OUTW, TS * BC), f32, kind="ExternalOutput")

    with tile.TileContext(nc) as tc:
        with (
            tc.tile_pool(name="singles", bufs=1) as singles,
            tc.tile_pool(name="ypool", bufs=4) as ypool,
            tc.tile_pool(name="stage", bufs=2) as stage,
            tc.tile_pool(name="psum", bufs=4, space="PSUM") as psum_pool,
        ):
            la_sb = singles.tile([NS, TS * OUTW], f32)
            nc.sync.dma_start(out=la_sb, in_=LA[:, :])
            lb_sb = singles.tile([MS, TS * OUTW], f32)
            nc.sync.dma_start(out=lb_sb, in_=LB[:, :])
            m0_sb = singles.tile([NS, BC], f32)
            nc.sync.dma_start(out=m0_sb, in_=M0[:, :])

            prep_sb = singles.tile([128, CS_COL], f32)
            srep_sb = singles.tile([128, CM_COL], f32)
            for k in range(REP):
                nc.sync.dma_start(out=prep_sb[8 * k : 8 * (k + 1), :], in_=PREP[:, :])
                nc.sync.dma_start(out=srep_sb[8 * k : 8 * (k + 1), :], in_=SREP[:, :])

            # Broadcast-write the shared covariance path: 16 x 3.27MB + 16 x
            # 0.82MB contiguous DMAs (each batch-row segment is one partition's
            # contiguous 25.6KB/6.4KB span).
            for g in range(N_GROUPS):
                nc.sync.dma_start(out=CS[g, :, :], in_=prep_sb[:, :])
                nc.sync.dma_start(out=CM[g, :, :], in_=srep_sb[:, :])

            # Batched mean recursion: per step two accumulating matmuls
            # produce [m_t ; mu_t] (24 x 256) in PSUM, one DVE copy lands it
            # in the staging tile that both feeds step t+1 and DMAs out.
            m_prev = m0_sb[:, :]
            st = None
            for s in range(TS):
                k = s % CHUNK_T
                if k == 0:
                    t0 = s
                    cT = min(CHUNK_T, TS - t0)
                    st = stage.tile([OUTW, CHUNK_T * BC], f32)
                if s % YCHUNK == 0:
                    yc = min(YCHUNK, TS - s)
                    y_sb = ypool.tile([MS, YCHUNK * BC], f32)
                    nc.sync.dma_start(
                        out=y_sb[:, : yc * BC],
                        in_=Y[:, s * BC : (s + yc) * BC],
                    )
                ps = psum_pool.tile([OUTW, BC], f32)
                nc.tensor.matmul(
                    ps[:, :],
                    lhsT=la_sb[:, s * OUTW : (s + 1) * OUTW],
                    rhs=m_prev,
                    start=True,
                    stop=False,
                )
                nc.tensor.matmul(
                    ps[:, :],
                    lhsT=lb_sb[:, s * OUTW : (s + 1) * OUTW],
                    rhs=y_sb[:, (s % YCHUNK) * BC : (s % YCHUNK + 1) * BC],
                    start=False,
                    stop=True,
                )
                nc.vector.tensor_copy(st[:, k * BC : (k + 1) * BC], ps[:, :])
                m_prev = st[:NS, k * BC : (k + 1) * BC]
                if k == cT - 1 or s == TS - 1:
                    nc.sync.dma_start(
                        out=MR[:, t0 * BC : (t0 + cT) * BC],
                        in_=st[:, : cT * BC],
                    )

    nc.compile()
    return nc


def _get_nc():
    if "nc" not in _CACHE:
        _CACHE["nc"] = _build_nc()
    return _CACHE["nc"]


def kernel(input, init_state_mean, init_log_diag, init_off_diag, F, H, Q, R):
    from concourse.bass_utils import run_bass_kernel_spmd

    inp = np.asarray(input, np.float32)
    pre = _host_precompute(
        init_state_mean, init_log_diag, init_off_diag, F, H, Q, R
    )
    nc = _get_nc()

    shared = {k: pre[k] for k in ("LA", "LB", "PREP", "SREP", "M0")}
    in_maps = []
    for c in range(N_CORES):
        sl = inp[c * BC : (c + 1) * BC, :TS, :]           # (BC, TS, MS)
        Yc = np.ascontiguousarray(sl.transpose(2, 1, 0)).reshape(MS, TS * BC)
        in_maps.append({**shared, "Y": Yc})

    trace = bool(int(os.environ.get("KF_TRACE", "0")))
    res = run_bass_kernel_spmd(
        nc, in_maps, core_ids=list(range(N_CORES)), trace=trace
    )
    kernel.last_exec_time_ns = res.exec_time_ns
    kernel.last_results = res

    state_means = np.empty((B, T, NS), np.float32)
    state_covs = np.empty((B, T, NS, NS), np.float32)
    meas_means = np.empty((B, T, MS), np.float32)
    meas_covs = np.empty((B, T, MS, MS), np.float32)
    state_means[:, 0, :] = pre["m0"]
    meas_means[:, 0, :] = pre["mu0"]
    for c in range(N_CORES):
        r = res.results[c]
        b0 = c * BC
        state_covs[b0 : b0 + BC] = r["CS"].reshape(BC, T, NS, NS)
        meas_covs[b0 : b0 + BC] = r["CM"].reshape(BC, T, MS, MS)
        mr = r["MR"]
        state_means[b0 : b0 + BC, 1:, :] = (
            mr[:NS].reshape(NS, TS, BC).transpose(2, 1, 0)
        )
        meas_means[b0 : b0 + BC, 1:, :] = (
            mr[NS:].reshape(MS, TS, BC).transpose(2, 1, 0)
        )
    return state_means, state_covs, meas_means, meas_covs


# revision 7
# speedup vs baseline: 2.0491x; 2.0491x over previous
"""Trainium2 Bass kernel for nn_KalmanFilter (B=2048, T=200, NS=16, MS=8).

Key structure: the Kalman covariance recursion (P_t, S_t, K_t, Sig_t) is
data-independent and the initial covariance is shared by every series, so the
whole covariance path is identical across the batch.  The host computes that
tiny 199-step recursion once (a few hundred KB of derived parameters); the
device then does the actual data-dependent work:

  * the batched mean recursion, reformulated as a chunked scan: for each chunk
    of L=16 steps, the stacked outputs [m_{t+1};mu_{t+1}] for the whole chunk
    are  G_c @ Y_chunk + Phi_c @ m_chunk_start  - two accumulating matmuls per
    96-row output tile (G_c, Phi_c are host-precomputed products of the
    per-step transition/gain matrices).  Only the tiny chunk-boundary state
    recursion (13 hops) is serial.
  * writing the full ~563MB of outputs, which is what makes this problem
    memory-bound: the covariance outputs are broadcast-writes of the shared
    200-step path, done as large (0.8-3.3MB) contiguous DMAs from replicated
    SBUF tiles.

Sharding: data-parallel over batch, 256 series per core on 8 cores.
"""

import os

import numpy as np

NS, MS = 16, 8
B, T = 2048, 200
N_CORES = 8
BC = B // N_CORES          # 256 series per core
TS = T - 1                 # 199 recursion steps
OUTW = NS + MS             # 24 stacked output rows (m_t ; mu_t)

L = 16                     # steps per scan chunk
NCH = (TS + L - 1) // L    # 13 chunks
JT = 4                     # steps per output tile (96 = JT*OUTW psum rows)

# Covariance broadcast-write geometry: one DMA writes `GROUP_ROWS` batch rows
# of the (T,NS,NS) / (T,MS,MS) path from a [128, *] SBUF tile.
N_GROUPS = 16
GROUP_ROWS = BC // N_GROUPS            # 16 batch rows per DMA
CS_ROW = T * NS * NS                   # 51200 floats per batch row
CM_ROW = T * MS * MS                   # 12800 floats per batch row
CS_COL = GROUP_ROWS * CS_ROW // 128    # 6400 floats per partition
CM_COL = GROUP_ROWS * CM_ROW // 128    # 1600 floats per partition
REP = 128 * CS_COL // CS_ROW           # 16 partition-replicas of the 8-row base

_CACHE = {}


def _host_precompute(init_state_mean, init_log_diag, init_off_diag, F, H, Q, R):
    """Mirror the reference's float32 covariance recursion; derive the chunked
    scan operators for the device mean recursion."""
    f32 = np.float32
    F = np.asarray(F, f32)
    H = np.asarray(H, f32)
    Q = np.asarray(Q, f32)
    R = np.asarray(R, f32)
    m0 = np.asarray(init_state_mean, f32)
    log_diag = np.asarray(init_log_diag, f32)
    off_diag = np.asarray(init_off_diag, f32)

    L0 = np.zeros((NS, NS), f32)
    L0[np.tril_indices(NS, -1)] = off_diag
    L0 += np.diag(np.exp(log_diag))
    P0 = (L0 @ L0.T).astype(f32)
    I = np.eye(NS, dtype=f32)

    P_seq = np.empty((T, NS, NS), f32)
    Sig_seq = np.empty((T, MS, MS), f32)
    A_seq = np.empty((TS + 1, NS, NS), f32)   # A_t, t=1..199
    B_seq = np.empty((TS + 1, NS, MS), f32)

    P = P0
    P_seq[0] = P0
    Sig_seq[0] = H @ P0 @ H.T + R
    for t in range(1, T):
        HP = H @ P                                   # (MS, NS)
        S = HP @ H.T + R
        K = np.linalg.solve(S, HP).T.astype(f32)     # (NS, MS) = P H^T S^-1
        ImKH = (I - K @ H).astype(f32)
        A_seq[t] = F @ ImKH
        B_seq[t] = F @ K
        P = (F @ (ImKH @ P) @ F.T + Q).astype(f32)
        P_seq[t] = P
        Sig_seq[t] = H @ P @ H.T + R

    # Chunked scan operators (accumulated in float64, stored float32):
    #   out[j*24+q] over chunk c = sum_l G[j,l] y_{t0+l} + Phi[j] m_{t0}
    # with G[j,l] = A_{t0+j+1}..A_{t0+l+2} B_{t0+l+1}, Phi[j] = prod A.
    A64 = A_seq.astype(np.float64)
    B64 = B_seq.astype(np.float64)
    H64 = H.astype(np.float64)
    LG = np.zeros((128, NCH * L * OUTW), f32)
    LP = np.zeros((NS, NCH * L * OUTW), f32)
    for c in range(NCH):
        t0 = c * L
        Lc = min(L, TS - t0)
        Pj = np.eye(NS)
        Ms = []                                     # M[l] = G[j, l], 16x8
        for j in range(Lc):
            t = t0 + j + 1
            Ms = [A64[t] @ m for m in Ms]
            Ms.append(B64[t])
            Pj = A64[t] @ Pj
            col = c * (L * OUTW) + j * OUTW
            LP[:, col : col + OUTW] = np.hstack(
                [Pj.T, (H64 @ Pj).T]
            ).astype(f32)
            for l in range(j + 1):
                W = np.vstack([Ms[l], H64 @ Ms[l]])  # (24, 8)
                LG[l * MS : (l + 1) * MS, col : col + OUTW] = W.T.astype(f32)

    mu0 = (m0 @ H.T).astype(f32)
    return {
        "LG": LG,
        "LP": LP,
        "PREP": P_seq.reshape(8, CS_COL).copy(),
        "SREP": Sig_seq.reshape(8, CM_COL).copy(),
        "M0": np.ascontiguousarray(np.broadcast_to(m0[:, None], (NS, BC))),
        "m0": m0,
        "mu0": mu0,
    }


def _build_nc():
    import concourse.bacc as bacc
    import concourse.bass as bass
    import concourse.tile as tile
    from concourse import mybir

    nc = bacc.Bacc("TRN2", debug=True)
    f32 = mybir.dt.float32
    CW = L * OUTW                      # 384 operator columns per chunk

    LG = nc.dram_tensor("LG", (128, NCH * CW), f32, kind="ExternalInput")
    LP = nc.dram_tensor("LP", (NS, NCH * CW), f32, kind="ExternalInput")
    PREP = nc.dram_tensor("PREP", (8, CS_COL), f32, kind="ExternalInput")
    SREP = nc.dram_tensor("SREP", (8, CM_COL), f32, kind="ExternalInput")
    M0 = nc.dram_tensor("M0", (NS, BC), f32, kind="ExternalInput")
    Y2 = nc.dram_tensor("Y2", (128, NCH * BC), f32, kind="ExternalInput")

    CS = nc.dram_tensor("CS", (N_GROUPS, 128, CS_COL), f32, kind="ExternalOutput")
    CM = nc.dram_tensor("CM", (N_GROUPS, 128, CM_COL), f32, kind="ExternalOutput")
    MR = nc.dram_tensor("MR", (TS, OUTW, BC), f32, kind="ExternalOutput")

    with tile.TileContext(nc) as tc:
        with (
            tc.tile_pool(name="singles", bufs=1) as singles,
            tc.tile_pool(name="ypool", bufs=3) as ypool,
            tc.tile_pool(name="mpool", bufs=3) as mpool,
            tc.tile_pool(name="stage", bufs=4) as stage,
            tc.tile_pool(name="psum", bufs=4, space="PSUM") as psum_pool,
            tc.tile_pool(name="psum_b", bufs=2, space="PSUM") as psum_b_pool,
        ):
            lg_sb = singles.tile([128, NCH * CW], f32)
            nc.sync.dma_start(out=lg_sb, in_=LG[:, :])
            lp_sb = singles.tile([NS, NCH * CW], f32)
            nc.sync.dma_start(out=lp_sb, in_=LP[:, :])
            m0_sb = singles.tile([NS, BC], f32)
            nc.sync.dma_start(out=m0_sb, in_=M0[:, :])

            prep_sb = singles.tile([128, CS_COL], f32)
            srep_sb = singles.tile([128, CM_COL], f32)
            # One broadcast DMA per path: read the 8-partition base REP times
            # into the 128-partition tile (src outer dim has stride 0).
            for base, dst in ((PREP, prep_sb), (SREP, srep_sb)):
                src = base[:, :]
                nc.sync.dma_start(
                    out=dst[:, :],
                    in_=bass.AP(tensor=src.tensor, offset=src.offset,
                                ap=[[0, REP]] + list(src.ap)),
                )

            # Broadcast-write the shared covariance path: 16 x 3.27MB + 16 x
            # 0.82MB contiguous DMAs (each batch-row segment is one partition's
            # contiguous 25.6KB/6.4KB span).
            for g in range(N_GROUPS):
                nc.sync.dma_start(out=CS[g, :, :], in_=prep_sb[:, :])
                nc.sync.dma_start(out=CM[g, :, :], in_=srep_sb[:, :])

            m_prev = m0_sb[:, :]
            for c in range(NCH):
                t0 = c * L
                Lc = min(L, TS - t0)
                y_sb = ypool.tile([128, BC], f32)
                nc.sync.dma_start(
                    out=y_sb, in_=Y2[:, c * BC : (c + 1) * BC]
                )
                # chunk-boundary state: rows (Lc-1)*24 .. +16 of the operators
                bcol = c * CW + (Lc - 1) * OUTW
                ps_b = psum_b_pool.tile([NS, BC], f32)
                nc.tensor.matmul(
                    ps_b[:, :],
                    lhsT=lg_sb[:, bcol : bcol + NS],
                    rhs=y_sb[:, :],
                    start=True,
                    stop=False,
                )
                nc.tensor.matmul(
                    ps_b[:, :],
                    lhsT=lp_sb[:, bcol : bcol + NS],
                    rhs=m_prev,
                    start=False,
                    stop=True,
                )
                m_next = mpool.tile([NS, BC], f32)
                nc.vector.tensor_copy(m_next[:, :], ps_b[:, :])

                # full output tiles: JT steps x 24 rows each
                for p in range((Lc + JT - 1) // JT):
                    nj = min(JT, Lc - p * JT)
                    rows = nj * OUTW
                    col = c * CW + p * JT * OUTW
                    ps = psum_pool.tile([JT * OUTW, BC], f32)
                    nc.tensor.matmul(
                        ps[:rows, :],
                        lhsT=lg_sb[:, col : col + rows],
                        rhs=y_sb[:, :],
                        start=True,
                        stop=False,
                    )
                    nc.tensor.matmul(
                        ps[:rows, :],
                        lhsT=lp_sb[:, col : col + rows],
                        rhs=m_prev,
                        start=False,
                        stop=True,
                    )
                    st = stage.tile([JT * OUTW, BC], f32)
                    nc.scalar.copy(st[:rows, :], ps[:rows, :])
                    nc.sync.dma_start(
                        out=MR[t0 + p * JT : t0 + p * JT + nj, :, :],
                        in_=st[:rows, :],
                    )
                m_prev = m_next[:, :]

    nc.compile()
    return nc


def _get_nc():
    if "nc" not in _CACHE:
        _CACHE["nc"] = _build_nc()
    return _CACHE["nc"]


def kernel(input, init_state_mean, init_log_diag, init_off_diag, F, H, Q, R):
    from concourse.bass_utils import run_bass_kernel_spmd

    inp = np.asarray(input, np.float32)
    pre = _host_precompute(
        init_state_mean, init_log_diag, init_off_diag, F, H, Q, R
    )
    nc = _get_nc()

    shared = {k: pre[k] for k in ("LG", "LP", "PREP", "SREP", "M0")}
    in_maps = []
    for c in range(N_CORES):
        sl = inp[c * BC : (c + 1) * BC, :TS, :]           # (BC, TS, MS)
        pad = np.zeros((BC, NCH * L, MS), np.float32)
        pad[:, :TS, :] = sl
        # Y2[j*8+i, c*256+b] = y[b, c*16+j, i]
        Yc = np.ascontiguousarray(
            pad.reshape(BC, NCH, L, MS)
            .transpose(1, 2, 3, 0)
            .reshape(NCH, L * MS, BC)
            .transpose(1, 0, 2)
            .reshape(L * MS, NCH * BC)
        )
        in_maps.append({**shared, "Y2": Yc})

    trace = bool(int(os.environ.get("KF_TRACE", "0")))
    res = run_bass_kernel_spmd(
        nc, in_maps, core_ids=list(range(N_CORES)), trace=trace
    )
    kernel.last_exec_time_ns = res.exec_time_ns
    kernel.last_results = res

    state_means = np.empty((B, T, NS), np.float32)
    state_covs = np.empty((B, T, NS, NS), np.float32)
    meas_means = np.empty((B, T, MS), np.float32)
    meas_covs = np.empty((B, T, MS, MS), np.float32)
    state_means[:, 0, :] = pre["m0"]
    meas_means[:, 0, :] = pre["mu0"]
    for c in range(N_CORES):
        r = res.results[c]
        b0 = c * BC
        state_covs[b0 : b0 + BC] = r["CS"].reshape(BC, T, NS, NS)
        meas_covs[b0 : b0 + BC] = r["CM"].reshape(BC, T, MS, MS)
        mr = r["MR"]                                      # (TS, OUTW, BC)
        state_means[b0 : b0 + BC, 1:, :] = mr[:, :NS, :].transpose(2, 0, 1)
        meas_means[b0 : b0 + BC, 1:, :] = mr[:, NS:, :].transpose(2, 0, 1)
    return state_means, state_covs, meas_means, meas_covs


# revision 9
# speedup vs baseline: 2.2221x; 1.0844x over previous
"""Trainium2 Bass kernel for nn_KalmanFilter (B=2048, T=200, NS=16, MS=8).

Key structure: the Kalman covariance recursion (P_t, S_t, K_t, Sig_t) is
data-independent and the initial covariance is shared by every series, so the
whole covariance path is identical across the batch.  The host computes that
tiny 199-step recursion once (a few hundred KB of derived parameters); the
device then does the actual data-dependent work:

  * the batched mean recursion, reformulated as a chunked scan: for each chunk
    of L=16 steps, the stacked outputs [m_{t+1};mu_{t+1}] for the whole chunk
    are  G_c @ Y_chunk + Phi_c @ m_chunk_start  - two accumulating matmuls per
    96-row output tile (G_c, Phi_c are host-precomputed products of the
    per-step transition/gain matrices).  Only the tiny chunk-boundary state
    recursion (13 hops) is serial.
  * writing the full ~563MB of outputs, which is what makes this problem
    memory-bound: the covariance outputs are broadcast-writes of the shared
    200-step path, done as large (0.8-3.3MB) contiguous DMAs from replicated
    SBUF tiles.

Sharding: data-parallel over batch, 256 series per core on 8 cores.
"""

import os

import numpy as np

NS, MS = 16, 8
B, T = 2048, 200
N_CORES = 8
BC = B // N_CORES          # 256 series per core
TS = T - 1                 # 199 recursion steps
OUTW = NS + MS             # 24 stacked output rows (m_t ; mu_t)

L = 16                     # steps per scan chunk
NCH = (TS + L - 1) // L    # 13 chunks
JT = 4                     # steps per output tile (96 = JT*OUTW psum rows)

# Covariance broadcast-write geometry: one DMA writes `GROUP_ROWS` batch rows
# of the (T,NS,NS) / (T,MS,MS) path from a [128, *] SBUF tile.
N_GROUPS = 16
GROUP_ROWS = BC // N_GROUPS            # 16 batch rows per DMA
CS_ROW = T * NS * NS                   # 51200 floats per batch row
CM_ROW = T * MS * MS                   # 12800 floats per batch row
CS_COL = GROUP_ROWS * CS_ROW // 128    # 6400 floats per partition
CM_COL = GROUP_ROWS * CM_ROW // 128    # 1600 floats per partition
REP = 128 * CS_COL // CS_ROW           # 16 partition-replicas of the 8-row base

_CACHE = {}


def _host_precompute(init_state_mean, init_log_diag, init_off_diag, F, H, Q, R):
    """Mirror the reference's float32 covariance recursion; derive the chunked
    scan operators for the device mean recursion."""
    f32 = np.float32
    F = np.asarray(F, f32)
    H = np.asarray(H, f32)
    Q = np.asarray(Q, f32)
    R = np.asarray(R, f32)
    m0 = np.asarray(init_state_mean, f32)
    log_diag = np.asarray(init_log_diag, f32)
    off_diag = np.asarray(init_off_diag, f32)

    L0 = np.zeros((NS, NS), f32)
    L0[np.tril_indices(NS, -1)] = off_diag
    L0 += np.diag(np.exp(log_diag))
    P0 = (L0 @ L0.T).astype(f32)
    I = np.eye(NS, dtype=f32)

    P_seq = np.empty((T, NS, NS), f32)
    Sig_seq = np.empty((T, MS, MS), f32)
    A_seq = np.empty((TS + 1, NS, NS), f32)   # A_t, t=1..199
    B_seq = np.empty((TS + 1, NS, MS), f32)

    P = P0
    P_seq[0] = P0
    Sig_seq[0] = H @ P0 @ H.T + R
    for t in range(1, T):
        HP = H @ P                                   # (MS, NS)
        S = HP @ H.T + R
        K = np.linalg.solve(S, HP).T.astype(f32)     # (NS, MS) = P H^T S^-1
        ImKH = (I - K @ H).astype(f32)
        A_seq[t] = F @ ImKH
        B_seq[t] = F @ K
        P = (F @ (ImKH @ P) @ F.T + Q).astype(f32)
        P_seq[t] = P
        Sig_seq[t] = H @ P @ H.T + R

    # Chunked scan operators (accumulated in float64, stored float32):
    #   out[j*24+q] over chunk c = sum_l G[j,l] y_{t0+l} + Phi[j] m_{t0}
    # with G[j,l] = A_{t0+j+1}..A_{t0+l+2} B_{t0+l+1}, Phi[j] = prod A.
    A64 = A_seq.astype(np.float64)
    B64 = B_seq.astype(np.float64)
    H64 = H.astype(np.float64)
    LG = np.zeros((128, NCH * L * OUTW), f32)
    LP = np.zeros((NS, NCH * L * OUTW), f32)
    for c in range(NCH):
        t0 = c * L
        Lc = min(L, TS - t0)
        Pj = np.eye(NS)
        Ms = []                                     # M[l] = G[j, l], 16x8
        for j in range(Lc):
            t = t0 + j + 1
            Ms = [A64[t] @ m for m in Ms]
            Ms.append(B64[t])
            Pj = A64[t] @ Pj
            col = c * (L * OUTW) + j * OUTW
            LP[:, col : col + OUTW] = np.hstack(
                [Pj.T, (H64 @ Pj).T]
            ).astype(f32)
            for l in range(j + 1):
                W = np.vstack([Ms[l], H64 @ Ms[l]])  # (24, 8)
                LG[l * MS : (l + 1) * MS, col : col + OUTW] = W.T.astype(f32)

    mu0 = (m0 @ H.T).astype(f32)
    return {
        "LG": LG,
        "LP": LP,
        "PREP": P_seq.reshape(8, CS_COL).copy(),
        "SREP": Sig_seq.reshape(8, CM_COL).copy(),
        "M0": np.ascontiguousarray(np.broadcast_to(m0[:, None], (NS, BC))),
        "m0": m0,
        "mu0": mu0,
    }


def _build_nc():
    import concourse.bacc as bacc
    import concourse.bass as bass
    import concourse.tile as tile
    from concourse import mybir

    nc = bacc.Bacc("TRN2", debug=True)
    f32 = mybir.dt.float32
    CW = L * OUTW                      # 384 operator columns per chunk

    LG = nc.dram_tensor("LG", (128, NCH * CW), f32, kind="ExternalInput")
    LP = nc.dram_tensor("LP", (NS, NCH * CW), f32, kind="ExternalInput")
    PREP = nc.dram_tensor("PREP", (8, CS_COL), f32, kind="ExternalInput")
    SREP = nc.dram_tensor("SREP", (8, CM_COL), f32, kind="ExternalInput")
    M0 = nc.dram_tensor("M0", (NS, BC), f32, kind="ExternalInput")
    Y2 = nc.dram_tensor("Y2", (128, NCH * BC), f32, kind="ExternalInput")

    CS = nc.dram_tensor("CS", (N_GROUPS, 128, CS_COL), f32, kind="ExternalOutput")
    CM = nc.dram_tensor("CM", (N_GROUPS, 128, CM_COL), f32, kind="ExternalOutput")
    MR = nc.dram_tensor("MR", (TS, OUTW, BC), f32, kind="ExternalOutput")

    with tile.TileContext(nc) as tc:
        with (
            tc.tile_pool(name="singles", bufs=1) as singles,
            tc.tile_pool(name="ypool", bufs=3) as ypool,
            tc.tile_pool(name="mpool", bufs=3) as mpool,
            tc.tile_pool(name="stage", bufs=4) as stage,
            tc.tile_pool(name="psum", bufs=4, space="PSUM") as psum_pool,
            tc.tile_pool(name="psum_b", bufs=2, space="PSUM") as psum_b_pool,
        ):
            lg_sb = singles.tile([128, NCH * CW], f32)
            nc.sync.dma_start(out=lg_sb, in_=LG[:, :])
            lp_sb = singles.tile([NS, NCH * CW], f32)
            nc.sync.dma_start(out=lp_sb, in_=LP[:, :])
            m0_sb = singles.tile([NS, BC], f32)
            nc.sync.dma_start(out=m0_sb, in_=M0[:, :])

            prep_sb = singles.tile([128, CS_COL], f32)
            srep_sb = singles.tile([128, CM_COL], f32)
            # One broadcast DMA per path: read the 8-partition base REP times
            # into the 128-partition tile (src outer dim has stride 0).
            for base, dst in ((PREP, prep_sb), (SREP, srep_sb)):
                src = base[:, :]
                nc.scalar.dma_start(
                    out=dst[:, :],
                    in_=bass.AP(tensor=src.tensor, offset=src.offset,
                                ap=[[0, REP]] + list(src.ap)),
                )

            # Broadcast-write the shared covariance path: 16 x 3.27MB + 16 x
            # 0.82MB contiguous DMAs (each batch-row segment is one partition's
            # contiguous 25.6KB/6.4KB span).  Issued on the ACT HWDGE ring so
            # the mean-scan's loads/stores on the SP ring aren't stuck behind
            # them in FIFO order.
            for g in range(N_GROUPS):
                nc.scalar.dma_start(out=CS[g, :, :], in_=prep_sb[:, :])
                nc.scalar.dma_start(out=CM[g, :, :], in_=srep_sb[:, :])

            m_prev = m0_sb[:, :]
            for c in range(NCH):
                t0 = c * L
                Lc = min(L, TS - t0)
                y_sb = ypool.tile([128, BC], f32)
                nc.sync.dma_start(
                    out=y_sb, in_=Y2[:, c * BC : (c + 1) * BC]
                )
                # chunk-boundary state: rows (Lc-1)*24 .. +16 of the operators
                bcol = c * CW + (Lc - 1) * OUTW
                ps_b = psum_b_pool.tile([NS, BC], f32)
                nc.tensor.matmul(
                    ps_b[:, :],
                    lhsT=lg_sb[:, bcol : bcol + NS],
                    rhs=y_sb[:, :],
                    start=True,
                    stop=False,
                )
                nc.tensor.matmul(
                    ps_b[:, :],
                    lhsT=lp_sb[:, bcol : bcol + NS],
                    rhs=m_prev,
                    start=False,
                    stop=True,
                )
                m_next = mpool.tile([NS, BC], f32)
                nc.vector.tensor_copy(m_next[:, :], ps_b[:, :])

                # full output tiles: JT steps x 24 rows each
                for p in range((Lc + JT - 1) // JT):
                    nj = min(JT, Lc - p * JT)
                    rows = nj * OUTW
                    col = c * CW + p * JT * OUTW
                    ps = psum_pool.tile([JT * OUTW, BC], f32)
                    nc.tensor.matmul(
                        ps[:rows, :],
                        lhsT=lg_sb[:, col : col + rows],
                        rhs=y_sb[:, :],
                        start=True,
                        stop=False,
                    )
                    nc.tensor.matmul(
                        ps[:rows, :],
                        lhsT=lp_sb[:, col : col + rows],
                        rhs=m_prev,
                        start=False,
                        stop=True,
                    )
                    st = stage.tile([JT * OUTW, BC], f32)
                    nc.vector.tensor_copy(st[:rows, :], ps[:rows, :])
                    nc.sync.dma_start(
                        out=MR[t0 + p * JT : t0 + p * JT + nj, :, :],
                        in_=st[:rows, :],
                    )
                m_prev = m_next[:, :]

    nc.compile()
    return nc


def _get_nc():
    if "nc" not in _CACHE:
        _CACHE["nc"] = _build_nc()
    return _CACHE["nc"]


def kernel(input, init_state_mean, init_log_diag, init_off_diag, F, H, Q, R):
    from concourse.bass_utils import run_bass_kernel_spmd

    inp = np.asarray(input, np.float32)
    pre = _host_precompute(
        init_state_mean, init_log_diag, init_off_diag, F, H, Q, R
    )
    nc = _get_nc()

    shared = {k: pre[k] for k in ("LG", "LP", "PREP", "SREP", "M0")}
    in_maps = []
    for c in range(N_CORES):
        sl = inp[c * BC : (c + 1) * BC, :TS, :]           # (BC, TS, MS)
        pad = np.zeros((BC, NCH * L, MS), np.float32)
        pad[:, :TS, :] = sl
        # Y2[j*8+i, c*256+b] = y[b, c*16+j, i]
        Yc = np.ascontiguousarray(
            pad.reshape(BC, NCH, L, MS)
            .transpose(1, 2, 3, 0)
            .reshape(NCH, L * MS, BC)
            .transpose(1, 0, 2)
            .reshape(L * MS, NCH * BC)
        )
        in_maps.append({**shared, "Y2": Yc})

    trace = bool(int(os.environ.get("KF_TRACE", "0")))
    res = run_bass_kernel_spmd(
        nc, in_maps, core_ids=list(range(N_CORES)), trace=trace
    )
    kernel.last_exec_time_ns = res.exec_time_ns
    kernel.last_results = res

    state_means = np.empty((B, T, NS), np.float32)
    state_covs = np.empty((B, T, NS, NS), np.float32)
    meas_means = np.empty((B, T, MS), np.float32)
    meas_covs = np.empty((B, T, MS, MS), np.float32)
    state_means[:, 0, :] = pre["m0"]
    meas_means[:, 0, :] = pre["mu0"]
    for c in range(N_CORES):
        r = res.results[c]
        b0 = c * BC
        state_covs[b0 : b0 + BC] = r["CS"].reshape(BC, T, NS, NS)
        meas_covs[b0 : b0 + BC] = r["CM"].reshape(BC, T, MS, MS)
        mr = r["MR"]                                      # (TS, OUTW, BC)
        state_means[b0 : b0 + BC, 1:, :] = mr[:, :NS, :].transpose(2, 0, 1)
        meas_means[b0 : b0 + BC, 1:, :] = mr[:, NS:, :].transpose(2, 0, 1)
    return state_means, state_covs, meas_means, meas_covs


# revision 13
# speedup vs baseline: 3.0300x; 1.3636x over previous
"""Trainium2 Bass kernel for nn_KalmanFilter (B=2048, T=200, NS=16, MS=8).

Key structure: the Kalman covariance recursion (P_t, S_t, K_t, Sig_t) is
data-independent and the initial covariance is shared by every series, so the
whole covariance path is identical across the batch.  The host computes that
tiny 199-step recursion once (a few hundred KB of derived parameters); the
device then does the actual data-dependent work:

  * the batched mean recursion, reformulated as a chunked scan: for each chunk
    of L=16 steps, the stacked outputs [m_{t+1};mu_{t+1}] for the whole chunk
    are  G_c @ Y_chunk + Phi_c @ m_chunk_start  - two accumulating matmuls per
    96-row output tile (G_c, Phi_c are host-precomputed products of the
    per-step transition/gain matrices).  Only the tiny chunk-boundary state
    recursion (13 hops) is serial.
  * writing the full ~563MB of outputs, which is what makes this problem
    memory-bound: the covariance outputs are broadcast-writes of the shared
    200-step path, done as large (0.8-3.3MB) contiguous DMAs from replicated
    SBUF tiles.

Sharding: data-parallel over batch, 256 series per core on 8 cores.
"""

import os

import numpy as np

NS, MS = 16, 8
B, T = 2048, 200
N_CORES = 8
BC = B // N_CORES          # 256 series per core
TS = T - 1                 # 199 recursion steps
OUTW = NS + MS             # 24 stacked output rows (m_t ; mu_t)

L = 16                     # steps per scan chunk
NCH = (TS + L - 1) // L    # 13 chunks
JT = 4                     # steps per output tile (96 = JT*OUTW psum rows)

# Covariance broadcast-write geometry: one DMA writes `GROUP_ROWS` batch rows
# of the (T,NS,NS) / (T,MS,MS) path from a [128, *] SBUF tile.
N_GROUPS = 16
GROUP_ROWS = BC // N_GROUPS            # 16 batch rows per DMA
CS_ROW = T * NS * NS                   # 51200 floats per batch row
CM_ROW = T * MS * MS                   # 12800 floats per batch row
CS_COL = GROUP_ROWS * CS_ROW // 128    # 6400 floats per partition
CM_COL = GROUP_ROWS * CM_ROW // 128    # 1600 floats per partition
REP = 128 * CS_COL // CS_ROW           # 16 partition-replicas of the 8-row base

_CACHE = {}


def _host_precompute(init_state_mean, init_log_diag, init_off_diag, F, H, Q, R):
    """Mirror the reference's float32 covariance recursion; derive the chunked
    scan operators for the device mean recursion."""
    f32 = np.float32
    F = np.asarray(F, f32)
    H = np.asarray(H, f32)
    Q = np.asarray(Q, f32)
    R = np.asarray(R, f32)
    m0 = np.asarray(init_state_mean, f32)
    log_diag = np.asarray(init_log_diag, f32)
    off_diag = np.asarray(init_off_diag, f32)

    L0 = np.zeros((NS, NS), f32)
    L0[np.tril_indices(NS, -1)] = off_diag
    L0 += np.diag(np.exp(log_diag))
    P0 = (L0 @ L0.T).astype(f32)
    I = np.eye(NS, dtype=f32)

    P_seq = np.empty((T, NS, NS), f32)
    Sig_seq = np.empty((T, MS, MS), f32)
    A_seq = np.empty((TS + 1, NS, NS), f32)   # A_t, t=1..199
    B_seq = np.empty((TS + 1, NS, MS), f32)

    P = P0
    P_seq[0] = P0
    Sig_seq[0] = H @ P0 @ H.T + R
    for t in range(1, T):
        HP = H @ P                                   # (MS, NS)
        S = HP @ H.T + R
        K = np.linalg.solve(S, HP).T.astype(f32)     # (NS, MS) = P H^T S^-1
        ImKH = (I - K @ H).astype(f32)
        A_seq[t] = F @ ImKH
        B_seq[t] = F @ K
        P = (F @ (ImKH @ P) @ F.T + Q).astype(f32)
        P_seq[t] = P
        Sig_seq[t] = H @ P @ H.T + R

    # Chunked scan operators (accumulated in float64, stored float32):
    #   out[j*24+q] over chunk c = sum_l G[j,l] y_{t0+l} + Phi[j] m_{t0}
    # with G[j,l] = A_{t0+j+1}..A_{t0+l+2} B_{t0+l+1}, Phi[j] = prod A.
    A64 = A_seq.astype(np.float64)
    B64 = B_seq.astype(np.float64)
    H64 = H.astype(np.float64)
    LG = np.zeros((128, NCH * L * OUTW), f32)
    LP = np.zeros((NS, NCH * L * OUTW), f32)
    for c in range(NCH):
        t0 = c * L
        Lc = min(L, TS - t0)
        Pj = np.eye(NS)
        Ms = []                                     # M[l] = G[j, l], 16x8
        for j in range(Lc):
            t = t0 + j + 1
            Ms = [A64[t] @ m for m in Ms]
            Ms.append(B64[t])
            Pj = A64[t] @ Pj
            col = c * (L * OUTW) + j * OUTW
            LP[:, col : col + OUTW] = np.hstack(
                [Pj.T, (H64 @ Pj).T]
            ).astype(f32)
            for l in range(j + 1):
                W = np.vstack([Ms[l], H64 @ Ms[l]])  # (24, 8)
                LG[l * MS : (l + 1) * MS, col : col + OUTW] = W.T.astype(f32)

    mu0 = (m0 @ H.T).astype(f32)
    return {
        "LG": LG,
        "LP": LP,
        "PREP": P_seq.reshape(8, CS_COL).copy(),
        "SREP": Sig_seq.reshape(8, CM_COL).copy(),
        "M0": np.ascontiguousarray(np.broadcast_to(m0[:, None], (NS, BC))),
        "m0": m0,
        "mu0": mu0,
    }


def _build_nc():
    import concourse.bacc as bacc
    import concourse.bass as bass
    import concourse.tile as tile
    from concourse import mybir

    nc = bacc.Bacc("TRN2", debug=True)
    f32 = mybir.dt.float32
    CW = L * OUTW                      # 384 operator columns per chunk

    LG = nc.dram_tensor("LG", (128, NCH * CW), f32, kind="ExternalInput")
    LP = nc.dram_tensor("LP", (NS, NCH * CW), f32, kind="ExternalInput")
    PREP = nc.dram_tensor("PREP", (8, CS_COL), f32, kind="ExternalInput")
    SREP = nc.dram_tensor("SREP", (8, CM_COL), f32, kind="ExternalInput")
    M0 = nc.dram_tensor("M0", (NS, BC), f32, kind="ExternalInput")
    Y2 = nc.dram_tensor("Y2", (128, NCH * BC), f32, kind="ExternalInput")

    CS = nc.dram_tensor("CS", (N_GROUPS, 128, CS_COL), f32, kind="ExternalOutput")
    CM = nc.dram_tensor("CM", (N_GROUPS, 128, CM_COL), f32, kind="ExternalOutput")
    MR = nc.dram_tensor("MR", (TS, OUTW, BC), f32, kind="ExternalOutput")

    with tile.TileContext(nc) as tc:
        with (
            tc.tile_pool(name="singles", bufs=1) as singles,
            tc.tile_pool(name="stage", bufs=4) as stage,
            tc.tile_pool(name="psum", bufs=4, space="PSUM") as psum_pool,
            tc.tile_pool(name="psum_b", bufs=2, space="PSUM") as psum_b_pool,
        ):
            lg_sb = singles.tile([128, NCH * CW], f32)
            nc.sync.dma_start(out=lg_sb, in_=LG[:, :])
            lp_sb = singles.tile([NS, NCH * CW], f32)
            nc.sync.dma_start(out=lp_sb, in_=LP[:, :])
            m0_sb = singles.tile([NS, BC], f32)
            nc.sync.dma_start(out=m0_sb, in_=M0[:, :])

            prep_sb = singles.tile([128, CS_COL], f32)
            srep_sb = singles.tile([128, CM_COL], f32)
            # One broadcast DMA per path: read the 8-partition base REP times
            # into the 128-partition tile (src outer dim has stride 0).
            for base, dst in ((PREP, prep_sb), (SREP, srep_sb)):
                src = base[:, :]
                nc.scalar.dma_start(
                    out=dst[:, :],
                    in_=bass.AP(tensor=src.tensor, offset=src.offset,
                                ap=[[0, REP]] + list(src.ap)),
                )

            # Broadcast-write the shared covariance path: 16 x 3.27MB + 16 x
            # 0.82MB contiguous DMAs (each batch-row segment is one partition's
            # contiguous 25.6KB/6.4KB span).  Issued on the ACT HWDGE ring so
            # the mean-scan's loads/stores on the SP ring aren't stuck behind
            # them in FIFO order.
            for g in range(N_GROUPS):
                nc.scalar.dma_start(out=CS[g, :, :], in_=prep_sb[:, :])
                nc.scalar.dma_start(out=CM[g, :, :], in_=srep_sb[:, :])

            # Phase 1 - dense, chain-free: gy = G @ Y for every output tile and
            # every chunk boundary.  These only need the inputs, so the PE
            # runs them back-to-back early (warming the HAM clock gate) while
            # the covariance DMAs drain on the other ring.
            ntiles = []
            y_tiles, gy_tiles, gyb_tiles = [], [], []
            for c in range(NCH):
                Lc = min(L, TS - c * L)
                ntiles.append((Lc + JT - 1) // JT)
                y_sb = singles.tile([128, BC], f32, tag=f"y{c}")
                nc.sync.dma_start(out=y_sb, in_=Y2[:, c * BC : (c + 1) * BC])
                y_tiles.append(y_sb)
            for c in range(NCH):
                Lc = min(L, TS - c * L)
                bcol = c * CW + (Lc - 1) * OUTW
                ps_b = psum_b_pool.tile([NS, BC], f32)
                nc.tensor.matmul(
                    ps_b[:, :],
                    lhsT=lg_sb[:, bcol : bcol + NS],
                    rhs=y_tiles[c][:, :],
                    start=True,
                    stop=True,
                )
                gyb = singles.tile([NS, BC], f32, tag=f"gyb{c}")
                nc.vector.tensor_copy(gyb[:, :], ps_b[:, :])
                gyb_tiles.append(gyb)
                for p in range(ntiles[c]):
                    nj = min(JT, Lc - p * JT)
                    rows = nj * OUTW
                    col = c * CW + p * JT * OUTW
                    ps = psum_pool.tile([JT * OUTW, BC], f32)
                    nc.tensor.matmul(
                        ps[:rows, :],
                        lhsT=lg_sb[:, col : col + rows],
                        rhs=y_tiles[c][:, :],
                        start=True,
                        stop=True,
                    )
                    gy = singles.tile([JT * OUTW, BC], f32, tag=f"gy{c}_{p}")
                    nc.vector.tensor_copy(gy[:rows, :], ps[:rows, :])
                    gy_tiles.append(gy)

            # Phase 2/3 - the tiny serial boundary chain, with per-chunk
            # output tiles (Phi @ m + gy) trailing it.
            m_prev = m0_sb[:, :]
            gy_it = iter(gy_tiles)
            for c in range(NCH):
                t0 = c * L
                Lc = min(L, TS - t0)
                bcol = c * CW + (Lc - 1) * OUTW
                ps_b = psum_b_pool.tile([NS, BC], f32)
                nc.tensor.matmul(
                    ps_b[:, :],
                    lhsT=lp_sb[:, bcol : bcol + NS],
                    rhs=m_prev,
                    start=True,
                    stop=True,
                )
                m_next = singles.tile([NS, BC], f32, tag=f"m{c}")
                nc.vector.tensor_add(
                    m_next[:, :], ps_b[:, :], gyb_tiles[c][:, :]
                )
                for p in range(ntiles[c]):
                    nj = min(JT, Lc - p * JT)
                    rows = nj * OUTW
                    col = c * CW + p * JT * OUTW
                    ps = psum_pool.tile([JT * OUTW, BC], f32)
                    nc.tensor.matmul(
                        ps[:rows, :],
                        lhsT=lp_sb[:, col : col + rows],
                        rhs=m_prev,
                        start=True,
                        stop=True,
                    )
                    gy = next(gy_it)
                    st = stage.tile([JT * OUTW, BC], f32)
                    nc.vector.tensor_add(
                        st[:rows, :], ps[:rows, :], gy[:rows, :]
                    )
                    nc.sync.dma_start(
                        out=MR[t0 + p * JT : t0 + p * JT + nj, :, :],
                        in_=st[:rows, :],
                    )
                m_prev = m_next[:, :]

    nc.compile()
    return nc


def _get_nc():
    if "nc" not in _CACHE:
        _CACHE["nc"] = _build_nc()
    return _CACHE["nc"]


def kernel(input, init_state_mean, init_log_diag, init_off_diag, F, H, Q, R):
    from concourse.bass_utils import run_bass_kernel_spmd

    inp = np.asarray(input, np.float32)
    pre = _host_precompute(
        init_state_mean, init_log_diag, init_off_diag, F, H, Q, R
    )
    nc = _get_nc()

    shared = {k: pre[k] for k in ("LG", "LP", "PREP", "SREP", "M0")}
    in_maps = []
    for c in range(N_CORES):
        sl = inp[c * BC : (c + 1) * BC, :TS, :]           # (BC, TS, MS)
        pad = np.zeros((BC, NCH * L, MS), np.float32)
        pad[:, :TS, :] = sl
        # Y2[j*8+i, c*256+b] = y[b, c*16+j, i]
        Yc = np.ascontiguousarray(
            pad.reshape(BC, NCH, L, MS)
            .transpose(1, 2, 3, 0)
            .reshape(NCH, L * MS, BC)
            .transpose(1, 0, 2)
            .reshape(L * MS, NCH * BC)
        )
        in_maps.append({**shared, "Y2": Yc})

    trace = bool(int(os.environ.get("KF_TRACE", "0")))
    res = run_bass_kernel_spmd(
        nc, in_maps, core_ids=list(range(N_CORES)), trace=trace
    )
    kernel.last_exec_time_ns = res.exec_time_ns
    kernel.last_results = res

    state_means = np.empty((B, T, NS), np.float32)
    state_covs = np.empty((B, T, NS, NS), np.float32)
    meas_means = np.empty((B, T, MS), np.float32)
    meas_covs = np.empty((B, T, MS, MS), np.float32)
    state_means[:, 0, :] = pre["m0"]
    meas_means[:, 0, :] = pre["mu0"]
    for c in range(N_CORES):
        r = res.results[c]
        b0 = c * BC
        state_covs[b0 : b0 + BC] = r["CS"].reshape(BC, T, NS, NS)
        meas_covs[b0 : b0 + BC] = r["CM"].reshape(BC, T, MS, MS)
        mr = r["MR"]                                      # (TS, OUTW, BC)
        state_means[b0 : b0 + BC, 1:, :] = mr[:, :NS, :].transpose(2, 0, 1)
        meas_means[b0 : b0 + BC, 1:, :] = mr[:, NS:, :].transpose(2, 0, 1)
    return state_means, state_covs, meas_means, meas_covs
